# revision 9
# baseline (speedup 1.0000x reference)
"""Trainium2 Bass kernel for tropical (min-plus) matmul:

    out[b, o] = min_i (W[o, i] + x[b, i]),   x: [512, 1024], W: [1024, 1024]

Strategy (v2, exact fp32 with candidate pruning, raw Bass):
  - Only i with x[b, i] <= min_i x[b, i] + 2*max|W| can attain the min
    (for any other i, W[o,i]+x[b,i] > minx + maxW >= the row minimum).
    With this data distribution the 128 smallest x per row are a strict
    superset of that candidate set (verified margin ~0.23 at K=128, actual
    candidate counts max ~91), so the min over the K=128 smallest-x columns
    is EXACT.
  - Host: per batch row, argpartition x for the K smallest indices; ship
    candidate indices [K, B/8] + values [K, B/8] per core (data-parallel
    shard over batch, 64 rows per core) plus W^T (replicated).
  - Device, per batch row b:
      1. gpsimd indirect DMA gathers the K candidate rows of W^T from DRAM
         into SBUF G [128 cand, 1024 o] (candidate on partitions).
      2. DVE tensor_scalar adds x[b, cand] (per-partition scalar, fp32 2x
         mode): S = G + xc.
      3. PE transposes S in 8 [128,128] blocks into PSUM P [128 o_loc, 8 j,
         128 cand].
      4. DVE tensor_reduce(min) over the candidate axis -> [128, 8] = out
         column b for all 1024 o, staged as ost[p, j, b].
  - Raw Bass (TileContext's multi-wait Drain and the fused ISA
    tensor_tensor_reduce do not compile on this toolchain).
  - `repeat` replays the body N times inside one NEFF so per-pass hardware
    time can be measured as a wall-clock delta (no NTFF profiling through
    this axon tunnel).
"""

import os
from contextlib import ExitStack

import numpy as np

import concourse.bass as bass
import concourse.mybir as mybir
from concourse.bass_utils import run_bass_kernel_spmd
from concourse.masks import make_identity

B, OUT, IN = 512, 1024, 1024
NCORES = 8
K = 128  # candidates per batch row
BSH = B // NCORES  # batch rows per core (64)
NJ = OUT // 128  # 8 o-blocks
F32 = mybir.dt.float32
I32 = mybir.dt.int32
AL = mybir.AluOpType

LAST_EXEC_NS = None


def _build_program(repeat: int = 1):
    nc = bass.Bass("TRN2", target_bir_lowering=False, debug=False)
    wT = nc.dram_tensor("wT", [IN, OUT], F32, kind="ExternalInput").ap()
    cidx = nc.dram_tensor("cidx", [K, BSH], I32, kind="ExternalInput").ap()
    xc = nc.dram_tensor("xc", [K, BSH], F32, kind="ExternalInput").ap()
    out = nc.dram_tensor("out", [128, NJ * BSH], F32, kind="ExternalOutput").ap()

    with ExitStack() as ctx:
        idxt = ctx.enter_context(nc.sbuf_tensor("idxt", [K, BSH], I32))
        xct = ctx.enter_context(nc.sbuf_tensor("xct", [K, BSH], F32))
        idt = ctx.enter_context(nc.sbuf_tensor("idt", [128, 128], F32))
        G = [
            ctx.enter_context(nc.sbuf_tensor(f"G{r}", [K, OUT], F32))
            for r in range(2)
        ]
        S = [
            ctx.enter_context(nc.sbuf_tensor(f"S{r}", [K, OUT], F32))
            for r in range(2)
        ]
        P = [
            ctx.enter_context(nc.psum_tensor(f"P{r}", [128, NJ * 128], F32))
            for r in range(2)
        ]
        ost = ctx.enter_context(nc.sbuf_tensor("ost", [128, NJ * BSH], F32))

        xsem = ctx.enter_context(nc.semaphore())
        isem = ctx.enter_context(nc.semaphore())
        gsem = ctx.enter_context(nc.semaphore())
        ssem = ctx.enter_context(nc.semaphore())
        tsem = ctx.enter_context(nc.semaphore())
        rsem = ctx.enter_context(nc.semaphore())
        osem = ctx.enter_context(nc.semaphore())
        block = ctx.enter_context(nc.Block())

        @block.sync
        def _(sync):
            sync.dma_start(idxt[:], cidx[:, :]).then_inc(xsem, 16)
            sync.dma_start(xct[:], xc[:, :]).then_inc(xsem, 16)
            for n in range(repeat):
                sync.wait_ge(rsem, BSH * (n + 1))
                sync.dma_start(out[:, :], ost[:]).then_inc(osem, 16)

        @block.gpsimd
        def _(gpsimd):
            make_identity(nc, idt[:])
            gpsimd.sem_inc(isem, 1)
            gpsimd.wait_ge(xsem, 16)
            for n in range(repeat):
                for b in range(BSH):
                    g = n * BSH + b
                    if g >= 2:
                        # G[g%2] was read by tensor_scalar pass g-2
                        gpsimd.wait_ge(ssem, g - 1)
                    gpsimd.indirect_dma_start(
                        out=G[g % 2][:],
                        out_offset=None,
                        in_=wT[:, :],
                        in_offset=bass.IndirectOffsetOnAxis(
                            ap=idxt[:, b : b + 1], axis=0
                        ),
                    ).then_inc(gsem, 16)

        @block.tensor
        def _(tensor):
            tensor.wait_ge(isem, 1)
            for n in range(repeat):
                for b in range(BSH):
                    g = n * BSH + b
                    tensor.wait_ge(ssem, g + 1)
                    if g >= 2:
                        tensor.wait_ge(rsem, g - 1)
                    for j in range(NJ):
                        nc.tensor.transpose(
                            out=P[g % 2][:, 128 * j : 128 * (j + 1)],
                            in_=S[g % 2][:, 128 * j : 128 * (j + 1)],
                            identity=idt[:],
                        ).then_inc(tsem, 1)

        @block.vector
        def _(vector):
            vector.wait_ge(xsem, 32)
            for n in range(repeat):
                for b in range(BSH):
                    g = n * BSH + b
                    vector.wait_ge(gsem, 16 * (g + 1))
                    if g >= 2:
                        # S[g%2] was read by PE transposes of pass g-2
                        vector.wait_ge(tsem, NJ * (g - 1))
                    nc.vector.tensor_scalar(
                        out=S[g % 2][:],
                        in0=G[g % 2][:],
                        scalar1=xct[:, b : b + 1],
                        scalar2=None,
                        op0=AL.add,
                    ).then_inc(ssem, 1)
                    if g >= 1:
                        vector.wait_ge(tsem, NJ * g)
                        bb = (g - 1) % BSH
                        nc.vector.tensor_reduce(
                            out=ost[:]
                            .rearrange("p (j b) -> p j b", b=BSH)[
                                :, :, bb : bb + 1
                            ],
                            in_=P[(g - 1) % 2][:].rearrange(
                                "p (j c) -> p j c", c=128
                            ),
                            axis=mybir.AxisListType.X,
                            op=AL.min,
                        ).then_inc(rsem, 1)
                glast = n * BSH + BSH - 1
                vector.wait_ge(tsem, NJ * (glast + 1))
                nc.vector.tensor_reduce(
                    out=ost[:]
                    .rearrange("p (j b) -> p j b", b=BSH)[
                        :, :, BSH - 1 : BSH
                    ],
                    in_=P[glast % 2][:].rearrange("p (j c) -> p j c", c=128),
                    axis=mybir.AxisListType.X,
                    op=AL.min,
                ).then_inc(rsem, 1)

    return nc


def _prep_host(x, W):
    """Candidate selection + input staging for each core."""
    wT = np.ascontiguousarray(W.T)  # [IN, OUT]
    part = np.argpartition(x, K - 1, axis=1)[:, :K]  # [B, K] smallest-x cols
    xcv = np.take_along_axis(x, part, axis=1)  # [B, K]
    in_maps = []
    for k in range(NCORES):
        sl = slice(BSH * k, BSH * (k + 1))
        in_maps.append(
            {
                "wT": wT,
                "cidx": np.ascontiguousarray(part[sl].T.astype(np.int32)),
                "xc": np.ascontiguousarray(xcv[sl].T.astype(np.float32)),
            }
        )
    return in_maps


def kernel(x: np.ndarray, W: np.ndarray) -> np.ndarray:
    x = np.ascontiguousarray(np.asarray(x, dtype=np.float32))
    W = np.ascontiguousarray(np.asarray(W, dtype=np.float32))
    assert x.shape == (B, IN) and W.shape == (OUT, IN)

    nc = _build_program()
    in_maps = _prep_host(x, W)
    res = run_bass_kernel_spmd(nc, in_maps, core_ids=list(range(NCORES)))
    # ost[p, j*BSH + b] = out[BSH*k + b, 128*j + p]
    cols = []
    for k in range(NCORES):
        ostk = res.results[k]["out"]  # [128, NJ*BSH]
        cols.append(ostk.reshape(128, NJ, BSH).transpose(2, 1, 0).reshape(BSH, OUT))
    return np.concatenate(cols, axis=0).astype(np.float32)


# revision 13
# speedup vs baseline: 16.8848x; 16.8848x over previous
"""Min-plus matmul, v4: instruction-count-minimal brute force.

Per-instruction overhead on this stack is ~60us, so the design uses as few,
as large, DVE instructions as possible:
  - shard over out_features: core k owns o in [128k, 128(k+1)); W shard
    [128, 1024] sits on partitions (one load, no broadcasts of W needed).
  - batch G=32 batch-rows per group: one DMA broadcasts x[32g:32g+32, :]
    across 128 partitions ([128, 32, 1024], 16MB); one tensor_tensor add
    with W repeated via a stride-0 middle AP dim (in-place over the x
    broadcast buffer); one 3D tensor_reduce(min) over i -> [128, 32]
    columns of ost.
  - 16 groups x ~3 instructions per core per pass.
"""

from contextlib import ExitStack

import numpy as np

import concourse.bass as bass
import concourse.mybir as mybir
from concourse.bass_utils import run_bass_kernel_spmd

B, OUT, IN = 512, 1024, 1024
NCORES = 8
OSH = OUT // NCORES  # 128
G = 32  # batch rows per group
NGRP = B // G  # 16
F32 = mybir.dt.float32
AL = mybir.AluOpType


def _build_program(repeat: int = 1):
    nc = bass.Bass("TRN2", target_bir_lowering=False, debug=False)
    x = nc.dram_tensor("x", [B, IN], F32, kind="ExternalInput").ap()
    w = nc.dram_tensor("w", [OSH, IN], F32, kind="ExternalInput").ap()
    out = nc.dram_tensor("out", [OSH, B], F32, kind="ExternalOutput").ap()

    with ExitStack() as ctx:
        wt = ctx.enter_context(nc.sbuf_tensor("wt", [128, IN], F32))
        xb = ctx.enter_context(nc.sbuf_tensor("xb", [128, G * IN], F32))
        ost = ctx.enter_context(nc.sbuf_tensor("ost", [128, B], F32))

        wsem = ctx.enter_context(nc.semaphore())
        bsem = ctx.enter_context(nc.semaphore())
        rsem = ctx.enter_context(nc.semaphore())
        osem = ctx.enter_context(nc.semaphore())
        block = ctx.enter_context(nc.Block())

        @block.sync
        def _(sync):
            sync.dma_start(wt[:], w[:, :]).then_inc(wsem, 16)
            for n in range(repeat):
                for g in range(NGRP):
                    t = n * NGRP + g
                    if t >= 1:
                        # single xb buffer: previous group's reduce must be done
                        sync.wait_ge(rsem, t)
                    src = x[G * g : G * (g + 1), :]
                    bc = bass.AP(src.tensor, src.offset, [[0, 128]] + src.ap)
                    sync.dma_start(xb[:], bc).then_inc(bsem, 16)
                sync.wait_ge(rsem, NGRP * (n + 1))
                sync.dma_start(out[:, :], ost[:]).then_inc(osem, 16)

        @block.vector
        def _(vector):
            vector.wait_ge(wsem, 16)
            for n in range(repeat):
                for g in range(NGRP):
                    t = n * NGRP + g
                    vector.wait_ge(bsem, 16 * (t + 1))
                    x3 = xb[:].rearrange("p (g i) -> p g i", g=G)
                    wrep = bass.AP(
                        wt[:].tensor,
                        wt[:].offset,
                        [wt[:].ap[0], [0, G], wt[:].ap[1]],
                    )
                    nc.vector.tensor_tensor(
                        out=x3, in0=wrep, in1=x3, op=AL.add
                    )
                    nc.vector.tensor_reduce(
                        out=ost[:, G * g : G * (g + 1)],
                        in_=x3,
                        axis=mybir.AxisListType.X,
                        op=AL.min,
                    ).then_inc(rsem, 1)

    return nc


def _prep_host(x, W):
    return [
        {"x": x, "w": np.ascontiguousarray(W[OSH * k : OSH * (k + 1), :])}
        for k in range(NCORES)
    ]


def kernel(x: np.ndarray, W: np.ndarray) -> np.ndarray:
    x = np.ascontiguousarray(np.asarray(x, dtype=np.float32))
    W = np.ascontiguousarray(np.asarray(W, dtype=np.float32))
    assert x.shape == (B, IN) and W.shape == (OUT, IN)

    nc = _build_program()
    in_maps = _prep_host(x, W)
    res = run_bass_kernel_spmd(nc, in_maps, core_ids=list(range(NCORES)))
    # out dram [OSH, B] per core: out[o_local, b] -> full[b, OSH*k + o_local]
    full = np.empty((B, OUT), dtype=np.float32)
    for k in range(NCORES):
        full[:, OSH * k : OSH * (k + 1)] = res.results[k]["out"].T
    return full


# revision 16
# speedup vs baseline: 21.9065x; 1.2974x over previous
"""Min-plus matmul, v4: instruction-count-minimal brute force.

Per-instruction overhead on this stack is ~60us, so the design uses as few,
as large, DVE instructions as possible:
  - shard over out_features: core k owns o in [128k, 128(k+1)); W shard
    [128, 1024] sits on partitions (one load, no broadcasts of W needed).
  - batch G=32 batch-rows per group: one DMA broadcasts x[32g:32g+32, :]
    across 128 partitions ([128, 32, 1024], 16MB); one tensor_tensor add
    with W repeated via a stride-0 middle AP dim (in-place over the x
    broadcast buffer); one 3D tensor_reduce(min) over i -> [128, 32]
    columns of ost.
  - 16 groups x ~3 instructions per core per pass.
"""

from contextlib import ExitStack

import numpy as np

import concourse.bass as bass
import concourse.mybir as mybir
from concourse.bass_utils import run_bass_kernel_spmd

B, OUT, IN = 512, 1024, 1024
NCORES = 8
OSH = OUT // NCORES  # 128
G = 45  # max batch rows per group (SBUF-bound: 45*4KB = 180KB/partition)
# non-uniform groups: 11x45 + 1x17 = 512 rows, 12 groups
GROUPS = [(s, min(G, B - s)) for s in range(0, B, G)]
F32 = mybir.dt.float32
AL = mybir.AluOpType


def _build_program(repeat: int = 1):
    nc = bass.Bass("TRN2", target_bir_lowering=False, debug=False)
    x = nc.dram_tensor("x", [B, IN], F32, kind="ExternalInput").ap()
    w = nc.dram_tensor("w", [OSH, IN], F32, kind="ExternalInput").ap()
    out = nc.dram_tensor("out", [OSH, B], F32, kind="ExternalOutput").ap()

    with ExitStack() as ctx:
        wt = ctx.enter_context(nc.sbuf_tensor("wt", [128, IN], F32))
        xb = ctx.enter_context(nc.sbuf_tensor("xb", [128, G * IN], F32))
        ost = ctx.enter_context(nc.sbuf_tensor("ost", [128, B], F32))

        wsem = ctx.enter_context(nc.semaphore())
        bsem = ctx.enter_context(nc.semaphore())
        rsem = ctx.enter_context(nc.semaphore())
        osem = ctx.enter_context(nc.semaphore())
        block = ctx.enter_context(nc.Block())

        @block.sync
        def _(sync):
            # W load counts into bsem so the first TT's single attached wait
            # covers both W and the first broadcast (one wait per instr max).
            sync.dma_start(wt[:], w[:, :]).then_inc(bsem, 16)
            for n in range(repeat):
                for g, (s, gl) in enumerate(GROUPS):
                    t = n * len(GROUPS) + g
                    src = x[s : s + gl, :]
                    bc = bass.AP(src.tensor, src.offset, [[0, 128]] + src.ap)
                    ins = sync.dma_start(xb[:, : gl * IN], bc)
                    if t >= 1:
                        # single xb buffer: previous group's reduce done
                        ins._wait_ge(rsem, t)
                    ins.then_inc(bsem, 16)
                sync.dma_start(out[:, :], ost[:])._wait_ge(
                    rsem, len(GROUPS) * (n + 1)
                ).then_inc(osem, 16)

        @block.vector
        def _(vector):
            for n in range(repeat):
                for g, (s, gl) in enumerate(GROUPS):
                    t = n * len(GROUPS) + g
                    x3 = xb[:, : gl * IN].rearrange(
                        "p (g i) -> p g i", g=gl
                    )
                    wrep = bass.AP(
                        wt[:].tensor,
                        wt[:].offset,
                        [wt[:].ap[0], [0, gl], wt[:].ap[1]],
                    )
                    nc.vector.tensor_tensor(
                        out=x3, in0=wrep, in1=x3, op=AL.add
                    )._wait_ge(bsem, 16 * (t + 2))
                    nc.vector.tensor_reduce(
                        out=ost[:, s : s + gl],
                        in_=x3,
                        axis=mybir.AxisListType.X,
                        op=AL.min,
                    ).then_inc(rsem, 1)

    return nc


def _prep_host(x, W):
    return [
        {"x": x, "w": np.ascontiguousarray(W[OSH * k : OSH * (k + 1), :])}
        for k in range(NCORES)
    ]


def kernel(x: np.ndarray, W: np.ndarray) -> np.ndarray:
    x = np.ascontiguousarray(np.asarray(x, dtype=np.float32))
    W = np.ascontiguousarray(np.asarray(W, dtype=np.float32))
    assert x.shape == (B, IN) and W.shape == (OUT, IN)

    nc = _build_program()
    in_maps = _prep_host(x, W)
    res = run_bass_kernel_spmd(nc, in_maps, core_ids=list(range(NCORES)))
    # out dram [OSH, B] per core: out[o_local, b] -> full[b, OSH*k + o_local]
    full = np.empty((B, OUT), dtype=np.float32)
    for k in range(NCORES):
        full[:, OSH * k : OSH * (k + 1)] = res.results[k]["out"].T
    return full


# revision 19
# speedup vs baseline: 25.7098x; 1.1736x over previous
"""Min-plus matmul, v4: instruction-count-minimal brute force.

Per-instruction overhead on this stack is ~60us, so the design uses as few,
as large, DVE instructions as possible:
  - shard over out_features: core k owns o in [128k, 128(k+1)); W shard
    [128, 1024] sits on partitions (one load, no broadcasts of W needed).
  - batch G=32 batch-rows per group: one DMA broadcasts x[32g:32g+32, :]
    across 128 partitions ([128, 32, 1024], 16MB); one tensor_tensor add
    with W repeated via a stride-0 middle AP dim (in-place over the x
    broadcast buffer); one 3D tensor_reduce(min) over i -> [128, 32]
    columns of ost.
  - 16 groups x ~3 instructions per core per pass.
"""

from contextlib import ExitStack

import numpy as np

import concourse.bass as bass
import concourse.mybir as mybir
from concourse.bass_utils import run_bass_kernel_spmd

B, OUT, IN = 512, 1024, 1024
NCORES = 8
OSH = OUT // NCORES  # 128
G = 47  # max batch rows per group (SBUF-bound: 47*4KB + 4KB wt = 192KB/part)
# non-uniform groups: 10x47 + 1x42 = 512 rows, 11 groups
GROUPS = [(s, min(G, B - s)) for s in range(0, B, G)]
F32 = mybir.dt.float32
AL = mybir.AluOpType


def _build_program(repeat: int = 1):
    nc = bass.Bass("TRN2", target_bir_lowering=False, debug=False)
    x = nc.dram_tensor("x", [B, IN], F32, kind="ExternalInput").ap()
    w = nc.dram_tensor("w", [OSH, IN], F32, kind="ExternalInput").ap()
    out = nc.dram_tensor("out", [OSH, B], F32, kind="ExternalOutput").ap()

    with ExitStack() as ctx:
        wt = ctx.enter_context(nc.sbuf_tensor("wt", [128, IN], F32))
        xb = ctx.enter_context(nc.sbuf_tensor("xb", [128, G * IN], F32))
        ost = ctx.enter_context(nc.sbuf_tensor("ost", [128, B], F32))

        wsem = ctx.enter_context(nc.semaphore())
        bsem = ctx.enter_context(nc.semaphore())
        rsem = ctx.enter_context(nc.semaphore())
        osem = ctx.enter_context(nc.semaphore())
        block = ctx.enter_context(nc.Block())

        @block.sync
        def _(sync):
            # W load counts into bsem so the first TT's single attached wait
            # covers both W and the first broadcast (one wait per instr max).
            sync.dma_start(wt[:], w[:, :]).then_inc(bsem, 16)
            for n in range(repeat):
                for g, (s, gl) in enumerate(GROUPS):
                    t = n * len(GROUPS) + g
                    src = x[s : s + gl, :]
                    bc = bass.AP(src.tensor, src.offset, [[0, 128]] + src.ap)
                    ins = sync.dma_start(xb[:, : gl * IN], bc)
                    if t >= 1:
                        # single xb buffer: previous group's reduce done
                        ins._wait_ge(rsem, t)
                    ins.then_inc(bsem, 16)
                sync.dma_start(out[:, :], ost[:])._wait_ge(
                    rsem, len(GROUPS) * (n + 1)
                ).then_inc(osem, 16)

        @block.vector
        def _(vector):
            for n in range(repeat):
                for g, (s, gl) in enumerate(GROUPS):
                    t = n * len(GROUPS) + g
                    x3 = xb[:, : gl * IN].rearrange(
                        "p (g i) -> p g i", g=gl
                    )
                    wrep = bass.AP(
                        wt[:].tensor,
                        wt[:].offset,
                        [wt[:].ap[0], [0, gl], wt[:].ap[1]],
                    )
                    nc.vector.tensor_tensor(
                        out=x3, in0=wrep, in1=x3, op=AL.add
                    )._wait_ge(bsem, 16 * (t + 2))
                    nc.vector.tensor_reduce(
                        out=ost[:, s : s + gl],
                        in_=x3,
                        axis=mybir.AxisListType.X,
                        op=AL.min,
                    ).then_inc(rsem, 1)

    return nc


def _prep_host(x, W):
    return [
        {"x": x, "w": np.ascontiguousarray(W[OSH * k : OSH * (k + 1), :])}
        for k in range(NCORES)
    ]


def kernel(x: np.ndarray, W: np.ndarray) -> np.ndarray:
    x = np.ascontiguousarray(np.asarray(x, dtype=np.float32))
    W = np.ascontiguousarray(np.asarray(W, dtype=np.float32))
    assert x.shape == (B, IN) and W.shape == (OUT, IN)

    nc = _build_program()
    in_maps = _prep_host(x, W)
    res = run_bass_kernel_spmd(nc, in_maps, core_ids=list(range(NCORES)))
    # out dram [OSH, B] per core: out[o_local, b] -> full[b, OSH*k + o_local]
    full = np.empty((B, OUT), dtype=np.float32)
    for k in range(NCORES):
        full[:, OSH * k : OSH * (k + 1)] = res.results[k]["out"].T
    return full
